# revision 22
# baseline (speedup 1.0000x reference)
"""Trainium2 Bass kernel for a 2-layer stacked bidirectional LSTM.

Problem (hardcoded): B=64, T=512, D=512, H=512, 2 BiLSTM layers,
Keras gate order [i, f, g, o], sigmoid recurrent activation, tanh cell
activation, merge_mode='concat'.

Sharding: 8 cores = 2 directions x 4 batch quarters (B'=16 per core).

v2 design (all cores run the identical SPMD program, ALL in LOCAL time):
  - Host pre-reverses x in time for backward cores, packs weights with the
    gate order permuted to [i, f, o, g] (so sigmoid gates are contiguous),
    swaps W1's row blocks per direction (own-direction rows first), and
    un-reverses the output for backward cores.
  - Recurrence is fully unrolled in python (no hardware loops): 16 chunks
    of 32 steps per layer. zx (input projections) live only in SBUF; the
    projection matmuls for chunk cc+1 are interleaved between the
    recurrence steps of chunk cc so the PE stays dense (HAM warm) and has
    no DRAM zx roundtrip.
  - Gate PSUM: one bank holds the 12 sigmoid-gate m-chunks, another the 4
    tanh m-chunks (m-outer, k-inner accumulation groups). The sigmoid
    z-add + activation overlap the tanh-block matmuls (different banks).
  - Per step tail (critical path): tanh-z-add -> tanh -> ig -> c -> tanh(c)
    -> h, with f*c computed early in parallel.
  - Layer 0 h sequence goes to DRAM (local time), pairwise AllGather with
    the partner core, layer-1 projection reads own half straight and the
    partner half chunk-reversed (static) + step-reversed via gpsimd
    copies (negative-stride DMAs are rejected by walrus).
"""

import numpy as np
import ml_dtypes

import concourse.bass as bass
import concourse.mybir as mybir
import concourse.tile as tile
from concourse.bass import ds, ts
from concourse.bass_utils import run_bass_kernel_spmd

BF16 = mybir.dt.bfloat16
F32 = mybir.dt.float32
I32 = mybir.dt.int32
AF = mybir.ActivationFunctionType
ALU = mybir.AluOpType

# Problem dims (full size)
B_FULL, T_FULL, D_FULL, H_FULL = 64, 512, 512, 512
N_CORES = 8
N_Q = 4   # batch quarters; cores 2q (fwd) and 2q+1 (bwd) handle quarter q
CH = 32   # recurrence steps per chunk (CH*BQ = 512 tokens per chunk)

_MAXW = 1  # max sem-waits per instruction accepted by this walrus

# PE "heater" matmuls per recurrence step: independent matmuls into a dead
# PSUM bank that fill the PE-idle window while the serial gate/cell tail
# computes, so the HAM clock gate stays at K=8/8 (2.4 GHz) instead of
# oscillating back to 1.2 GHz between matmul bursts.
N_HEAT = 0
HEAT_N = 256


def _fix_walrus_compat(nc):
    """Adapt Tile-emitted IR to the deployed walrus:
    - drop EVENT_SEMAPHORE_RANGE_CLEAR (InstISA) kernel-tail cleanup,
    - split instructions carrying more than _MAXW semaphore waits into
      leading single-wait NOPs.
    """
    n_split = n_drop = 0
    for bb in nc.main_func.blocks:
        insts = bb.instructions
        out = []
        for inst in insts:
            if isinstance(inst, mybir.InstISA):
                n_drop += 1
                continue
            si = inst.sync_info
            if si is not None and len(si.on_wait) > _MAXW:
                waits = list(si.on_wait)
                extra, keep = waits[:-_MAXW], waits[-_MAXW:]
                for w in extra:
                    nop = mybir.InstNoOp(
                        name=nc.get_next_instruction_name(), ins=[], outs=[])
                    nop.engine = inst.engine
                    nop.sync_info = mybir.SyncInfo(on_wait=[w], on_update=[])
                    out.append(nop)
                    n_split += 1
                inst.sync_info = mybir.SyncInfo(
                    on_wait=keep, on_update=list(si.on_update))
            out.append(inst)
        insts[:] = out
    return n_drop, n_split


def build_program(T=T_FULL, BQ=B_FULL // N_Q, D=D_FULL, H=H_FULL,
                  single_core=False):
    G = 4 * H
    KD0 = D // 128           # k-chunks, layer-0 projection
    KD1 = 2 * H // 128       # k-chunks, layer-1 projection
    KH = H // 128            # k-chunks, recurrence
    MC = G // 128            # m-chunks of the gate dim
    MS = 3 * (MC // 4)       # sigmoid m-chunks (i, f, o)
    MT = MC // 4             # tanh m-chunks (g)
    NT = T // CH             # chunks per layer
    TW = CH * BQ             # tokens per chunk (= proj tile width)
    FH = KH * CH * BQ        # flattened h-chunk free size
    assert T % CH == 0 and D % 128 == 0 and H % 128 == 0

    nc = bass.Bass("TRN2", target_bir_lowering=False, debug=False,
                   num_devices=1 if single_core else N_CORES)

    # ---- I/O (all host-packed, local time, gate order [i,f,o,g]) ----
    x_in = nc.dram_tensor("x_in", [NT, 128, KD0 * TW], BF16,
                          kind="ExternalInput")
    w0 = nc.dram_tensor("w0", [128, KD0 * G], BF16, kind="ExternalInput")
    u0 = nc.dram_tensor("u0", [128, KH * G], BF16, kind="ExternalInput")
    b0 = nc.dram_tensor("b0", [128, MC], F32, kind="ExternalInput")
    w1 = nc.dram_tensor("w1", [128, KD1 * G], BF16, kind="ExternalInput")
    u1 = nc.dram_tensor("u1", [128, KH * G], BF16, kind="ExternalInput")
    b1 = nc.dram_tensor("b1", [128, MC], F32, kind="ExternalInput")
    pflag = nc.dram_tensor("pflag", [1, 1], I32, kind="ExternalInput")
    ident = nc.dram_tensor("ident", [128, 128], BF16, kind="ExternalInput")
    h1out = nc.dram_tensor("h1out", [NT, 128, FH], BF16,
                           kind="ExternalOutput")

    groups = [[2 * q, 2 * q + 1] for q in range(N_Q)]

    with tile.TileContext(nc) as tc:
        consts = tc.alloc_tile_pool(name="consts", bufs=1)
        dram = tc.alloc_tile_pool(name="dram", bufs=1, space="DRAM")

        # weights / biases resident in SBUF for the whole kernel
        w0_sb = consts.tile([128, KD0, G], BF16)
        nc.sync.dma_start(w0_sb, w0.ap())
        u0_sb = consts.tile([128, KH, G], BF16)
        nc.sync.dma_start(u0_sb, u0.ap())
        w1_sb = consts.tile([128, KD1, G], BF16)
        nc.sync.dma_start(w1_sb, w1.ap())
        u1_sb = consts.tile([128, KH, G], BF16)
        nc.sync.dma_start(u1_sb, u1.ap())
        b0_sb = consts.tile([128, MC], F32)
        nc.sync.dma_start(b0_sb, b0.ap())
        b1_sb = consts.tile([128, MC], F32)
        nc.sync.dma_start(b1_sb, b1.ap())
        pflag_sb = consts.tile([1, 1], I32)
        nc.sync.dma_start(pflag_sb, pflag.ap())
        id_sb = consts.tile([128, 128], BF16)
        nc.sync.dma_start(id_sb, ident.ap())
        zero_h = consts.tile([128, KH, BQ], BF16)
        nc.vector.memset(zero_h, 0.0)
        # cell state + tanh(g) scratch per layer: X[:, 0:KH] = tanh(g),
        # X[:, KH:2KH] = c
        X0 = consts.tile([128, 2 * KH, BQ], F32, name="X0")
        X1 = consts.tile([128, 2 * KH, BQ], F32, name="X1")
        nc.vector.memset(X0, 0.0)
        nc.vector.memset(X1, 0.0)

        # DRAM scratch: layer-0 h sequence (local time, two halves so the
        # first AllGather can fire mid-recurrence) + AllGather results +
        # partner blocks (one dynamic-offset copy per half out of ag)
        HNT = NT // 2
        h0A = dram.tile([HNT, 128, FH], BF16)
        h0B = dram.tile([HNT, 128, FH], BF16)
        agA = dram.tile([2 * HNT, 128, FH], BF16)
        agB = dram.tile([2 * HNT, 128, FH], BF16)
        partA = dram.tile([HNT, 128, FH], BF16)
        partB = dram.tile([HNT, 128, FH], BF16)

        fvp = nc.values_load(pflag_sb[0:1, 0:1], min_val=0, max_val=1)

        # ---------------- pools ----------------
        xpool = tc.alloc_tile_pool(name="xpool", bufs=2)
        zxpool = tc.alloc_tile_pool(name="zxpool", bufs=2)
        hckpool = tc.alloc_tile_pool(name="hckpool", bufs=2)
        tailpool = tc.alloc_tile_pool(name="tailpool", bufs=2)
        ps_sig = tc.alloc_tile_pool(name="ps_sig", bufs=1, space="PSUM")
        ps_tanh = tc.alloc_tile_pool(name="ps_tanh", bufs=1, space="PSUM")
        pp = tc.alloc_tile_pool(name="pp", bufs=2, space="PSUM")
        ps_heat = tc.alloc_tile_pool(name="ps_heat", bufs=1, space="PSUM")

        psig = ps_sig.tile([128, MS, BQ], F32, name="psig")
        ptanh = ps_tanh.tile([128, MT, BQ], F32, name="ptanh")
        pheat = ps_heat.tile([128, HEAT_N], F32, name="pheat")

        def heaters(u_sb):
            for _ in range(N_HEAT):
                nc.tensor.matmul(pheat, u_sb[:, 0, 0:128],
                                 u_sb[:, 1, 0:HEAT_N], start=True, stop=True)

        # ---------------- helpers ----------------
        def load_x0(cc, xs):
            """x tile for layer-0 projection of chunk cc."""
            nc.sync.dma_start(
                xs, x_in.ap()[cc].rearrange("p (k t) -> p k t", k=KD0))

        def proj_group(m, w_sb, x_sb, KD, zx_sb, b_sb):
            """One projection m-group: 4|8 matmuls (N=TW) + biased copy."""
            ps = pp.tile([128, TW], F32, tag="pp")
            for k in range(KD):
                nc.tensor.matmul(ps, w_sb[:, k, ts(m, 128)], x_sb[:, k, :],
                                 start=(k == 0), stop=(k == KD - 1))
            nc.scalar.activation(
                zx_sb[:, :, m, :],
                ps.rearrange("p (j b) -> p j b", j=CH),
                AF.Identity, bias=b_sb[:, m:m + 1])

        def load_x1(cc, xs, pt):
            """Layer-1 proj inputs for chunk cc: own half straight from
            h0[cc]; partner half = part[NT-1-cc] (chunk-reversed)."""
            own = h0A[cc] if cc < HNT else h0B[cc - HNT]
            nc.sync.dma_start(
                xs[:, 0:KH, :], own.rearrange("p (k t) -> p k t", k=KH))
            pc = NT - 1 - cc
            psrc = partA[pc] if pc < HNT else partB[pc - HNT]
            nc.sync.dma_start(pt[:, 0, :], psrc)

        def reverse_steps(pt, xs, j_lo, j_hi):
            """Step-reverse partner h chunk into xs[:, KH:2KH, :]."""
            ptv = pt.rearrange("p one (k j b) -> p one k j b", k=KH, j=CH)
            for j in range(j_lo, j_hi):
                nc.gpsimd.tensor_copy(
                    xs[:, KH:2 * KH, ts(CH - 1 - j, BQ)],
                    ptv[:, 0, :, j, :])

        # ---------------- one recurrence step ----------------
        def step(u_sb, zx_sb, j, rhs_tile, rhs_j, h_ck, X):
            """rhs = rhs_tile[:, k, rhs_j, :] ([128,KH,CH,BQ]) or zero_h
            ([128,KH,BQ]) when rhs_j is None."""
            def rhs(k):
                if rhs_j is None:
                    return rhs_tile[:, k, :]
                return rhs_tile[:, k, rhs_j, :]

            # zx goes into PSUM first via an identity matmul (start=True
            # sets has_written for the whole region), then the U.h m-groups
            # accumulate on top with start=False.
            nc.tensor.matmul(
                psig, id_sb,
                zx_sb[:, j, 0:MS, :].rearrange("p m b -> p (m b)"),
                start=True, stop=False, skip_group_check=True)
            for m in range(MS):
                for k in range(KH):
                    nc.tensor.matmul(psig[:, m, :], u_sb[:, k, ts(m, 128)],
                                     rhs(k), start=False,
                                     stop=(m == MS - 1 and k == KH - 1),
                                     skip_group_check=True)
            # sigmoid block complete: activation reads PSUM directly and
            # overlaps the tanh-block matmuls (different PSUM banks)
            S = tailpool.tile([128, MS, BQ], F32, tag="S")
            nc.scalar.activation(S, psig, AF.Sigmoid)
            nc.tensor.matmul(
                ptanh, id_sb,
                zx_sb[:, j, MS:MC, :].rearrange("p m b -> p (m b)"),
                start=True, stop=False, skip_group_check=True)
            for m in range(MT):
                for k in range(KH):
                    nc.tensor.matmul(ptanh[:, m, :],
                                     u_sb[:, k, ts(MS + m, 128)],
                                     rhs(k), start=False,
                                     stop=(m == MT - 1 and k == KH - 1),
                                     skip_group_check=True)
            # f*c early (off the critical chain): fc = S[f] * X[c]
            fc = tailpool.tile([128, KH, BQ], F32, tag="fc")
            nc.vector.tensor_tensor(
                fc, S[:, KH:2 * KH, :], X[:, KH:2 * KH, :], ALU.mult)
            # critical chain: tanh(g) -> ig -> c -> tanh(c) -> h
            nc.scalar.activation(X[:, 0:KH, :], ptanh, AF.Tanh)
            ig = tailpool.tile([128, KH, BQ], F32, tag="ig")
            nc.vector.tensor_tensor(
                ig, S[:, 0:KH, :], X[:, 0:KH, :], ALU.mult)
            nc.vector.tensor_tensor(X[:, KH:2 * KH, :], ig, fc, ALU.add)
            th = tailpool.tile([128, KH, BQ], F32, tag="th")
            nc.scalar.activation(th, X[:, KH:2 * KH, :], AF.Tanh)
            nc.vector.tensor_tensor(
                h_ck[:, :, j, :], S[:, 2 * KH:3 * KH, :], th, ALU.mult)

        # ================= layer 0 =================
        x_sb = {}
        x_sb[0] = xpool.tile([128, KD0, TW], BF16, tag="x0", name="xsb0")
        load_x0(0, x_sb[0])
        zx = {}
        zx[0] = zxpool.tile([128, CH, MC, BQ], BF16, tag="zx", name="zx0")
        for m in range(MC):
            proj_group(m, w0_sb, x_sb[0], KD0, zx[0], b0_sb)
        x_sb[1] = xpool.tile([128, KD0, TW], BF16, tag="x0", name="xsb1")
        load_x0(1, x_sb[1])

        h_ck_prev = None
        for cc in range(NT):
            h_ck = hckpool.tile([128, KH, CH, BQ], BF16, tag="hck0")
            if cc + 1 < NT:
                zx[cc + 1] = zxpool.tile([128, CH, MC, BQ], BF16, tag="zx", name=f"zx{cc+1}")
            for j in range(CH):
                if j == 0:
                    if cc == 0:
                        step(u0_sb, zx[cc], j, zero_h, None, h_ck, X0)
                    else:
                        step(u0_sb, zx[cc], j, h_ck_prev, CH - 1, h_ck, X0)
                else:
                    step(u0_sb, zx[cc], j, h_ck, j - 1, h_ck, X0)
                if j == 1 and cc + 2 < NT:
                    x_sb[cc + 2] = xpool.tile([128, KD0, TW], BF16, tag="x0", name=f"xsb{cc+2}")
                    load_x0(cc + 2, x_sb[cc + 2])
                if j % 2 == 0 and cc + 1 < NT:
                    proj_group(j // 2, w0_sb, x_sb[cc + 1], KD0,
                               zx[cc + 1], b0_sb)
                heaters(u0_sb)
            dst = h0A[cc] if cc < HNT else h0B[cc - HNT]
            nc.sync.dma_start(dst, h_ck.rearrange("p k j b -> p (k j b)"))
            h_ck_prev = h_ck
            if cc == HNT - 1:
                # first-half AllGather overlaps the second half of rec0
                if single_core:
                    nc.sync.dma_start(agA[0:HNT], h0A)
                    nc.sync.dma_start(agA[HNT:2 * HNT], h0A)
                else:
                    nc.gpsimd.collective_compute(
                        "AllGather", ALU.bypass, replica_groups=groups,
                        ins=[h0A.opt()], outs=[agA.opt()])

        # ================= AllGather (second half) =================
        if single_core:
            nc.sync.dma_start(agB[0:HNT], h0B)
            nc.sync.dma_start(agB[HNT:2 * HNT], h0B)
        else:
            nc.gpsimd.collective_compute(
                "AllGather", ALU.bypass, replica_groups=groups,
                ins=[h0B.opt()], outs=[agB.opt()])
        # copy the partner's blocks out of the gathers with one
        # dynamic-offset DMA each (B first: proj1 chunk 0 needs partner's
        # last chunks); everything downstream is then static
        for ag, prt in ((agB, partB), (agA, partA)):
            poff = nc.s_assert_within(
                fvp * (HNT * 128 * FH), 0, HNT * 128 * FH)
            apg = ag[0]
            nc.sync.dma_start(
                prt.rearrange("c p f -> (c p) f"),
                bass.AP(tensor=apg.tensor, offset=apg.offset + poff,
                        ap=[[FH, HNT * 128], [1, FH]]))

        # ================= layer 1 =================
        x1 = {}
        pt = {}
        x1[0] = xpool.tile([128, KD1, TW], BF16, tag="x1", name="x1_0")
        pt[0] = xpool.tile([128, 1, FH], BF16, tag="pt", name="pt0")
        load_x1(0, x1[0], pt[0])
        reverse_steps(pt[0], x1[0], 0, CH)
        zx1 = {}
        zx1[0] = zxpool.tile([128, CH, MC, BQ], BF16, tag="zx", name="zx1_0")
        for m in range(MC):
            proj_group(m, w1_sb, x1[0], KD1, zx1[0], b1_sb)

        h_ck_prev = None
        for cc in range(NT):
            h_ck = hckpool.tile([128, KH, CH, BQ], BF16, tag="hck1")
            if cc + 1 < NT:
                zx1[cc + 1] = zxpool.tile([128, CH, MC, BQ], BF16, tag="zx", name=f"zx1_{cc+1}")
            for j in range(CH):
                if j == 0:
                    if cc == 0:
                        step(u1_sb, zx1[cc], j, zero_h, None, h_ck, X1)
                    else:
                        step(u1_sb, zx1[cc], j, h_ck_prev, CH - 1, h_ck, X1)
                else:
                    step(u1_sb, zx1[cc], j, h_ck, j - 1, h_ck, X1)
                if cc + 1 < NT:
                    if j == 0:
                        x1[cc + 1] = xpool.tile([128, KD1, TW], BF16,
                                                tag="x1", name=f"x1_{cc+1}")
                        pt[cc + 1] = xpool.tile([128, 1, FH], BF16, tag="pt", name=f"pt{cc+1}")
                        load_x1(cc + 1, x1[cc + 1], pt[cc + 1])
                    if 2 <= j < 10:
                        reverse_steps(pt[cc + 1], x1[cc + 1],
                                      (j - 2) * 4, (j - 1) * 4)
                    if 10 <= j < 26:
                        proj_group(j - 10, w1_sb, x1[cc + 1], KD1,
                                   zx1[cc + 1], b1_sb)
                heaters(u1_sb)
            nc.sync.dma_start(h1out.ap()[cc],
                              h_ck.rearrange("p k j b -> p (k j b)"))
            h_ck_prev = h_ck

        for p in (ps_heat, pp, ps_tanh, ps_sig, tailpool, hckpool, zxpool, xpool,
                  dram, consts):
            p.release()

    _fix_walrus_compat(nc)
    return nc


# gate permutation: Keras [i, f, g, o] -> kernel [i, f, o, g]
def _gate_perm(H):
    return np.concatenate([np.arange(0, H), np.arange(H, 2 * H),
                           np.arange(3 * H, 4 * H), np.arange(2 * H, 3 * H)])


def _pack_w(W, H):
    """[Din, G] -> [128, KD*G] bf16 with gate perm; row-major k-chunks."""
    bf = ml_dtypes.bfloat16
    Din, G = W.shape
    Wp = W[:, _gate_perm(H)]
    KD = Din // 128
    # [KD, 128, G] -> [128, KD, G]
    return np.ascontiguousarray(
        Wp.reshape(KD, 128, G).transpose(1, 0, 2).reshape(128, KD * G)
    ).astype(bf)


def _pack_b(b, H):
    MC = b.shape[0] // 128
    bp = b[_gate_perm(H)]
    return np.ascontiguousarray(bp.reshape(MC, 128).T).astype(np.float32)


def _prep_core_inputs(x, W0f, U0f, b0f, W0b, U0b, b0b,
                      W1f, U1f, b1f, W1b, U1b, b1b, T, BQ):
    """Host-side sharding: list of 8 input dicts (core = 2q + dir)."""
    bf = ml_dtypes.bfloat16
    B, _, D = x.shape
    H = U0f.shape[0]
    NT = T // CH
    KD0 = D // 128
    Wd = {0: (W0f, U0f, b0f, W1f, U1f, b1f),
          1: (W0b, U0b, b0b, W1b, U1b, b1b)}
    packed = {}
    for d in range(2):
        W0, U0, b0, W1, U1, b1 = Wd[d]
        # W1 rows: own-direction block first, partner block second
        if d == 0:
            W1o = W1
        else:
            W1o = np.concatenate([W1[H:2 * H], W1[0:H]], axis=0)
        packed[d] = {
            "w0": _pack_w(np.asarray(W0), H),
            "u0": _pack_w(np.asarray(U0), H),
            "b0": _pack_b(np.asarray(b0), H),
            "w1": _pack_w(np.asarray(W1o), H),
            "u1": _pack_w(np.asarray(U1), H),
            "b1": _pack_b(np.asarray(b1), H),
            "pflag": np.array([[1 - d]], dtype=np.int32),
            "ident": np.eye(128, dtype=np.float32).astype(bf),
        }
    in_maps = []
    for q in range(N_Q):
        xq = np.asarray(x[q * BQ:(q + 1) * BQ])      # [BQ, T, D]
        for d in range(2):
            xl = xq if d == 0 else xq[:, ::-1]
            # x_in[cc, p, kd, j*BQ+b] = xl[b, cc*CH+j, kd*128+p]
            xi = (xl.transpose(2, 1, 0)               # [D, T, BQ]
                  .reshape(KD0, 128, NT, CH, BQ)
                  .transpose(2, 1, 0, 3, 4)           # [NT, 128, KD0, CH, BQ]
                  .reshape(NT, 128, KD0 * CH * BQ))
            in_maps.append({
                "x_in": np.ascontiguousarray(xi).astype(bf),
                **packed[d],
            })
    return in_maps


def kernel(x, W0f, U0f, b0f, W0b, U0b, b0b,
           W1f, U1f, b1f, W1b, U1b, b1b):
    x = np.asarray(x, dtype=np.float32)
    B, T, D = x.shape
    H = U0f.shape[0]
    BQ = B // N_Q
    NT = T // CH
    KH = H // 128
    nc = build_program(T=T, BQ=BQ, D=D, H=H)
    in_maps = _prep_core_inputs(
        x, np.asarray(W0f), np.asarray(U0f), np.asarray(b0f),
        np.asarray(W0b), np.asarray(U0b), np.asarray(b0b),
        np.asarray(W1f), np.asarray(U1f), np.asarray(b1f),
        np.asarray(W1b), np.asarray(U1b), np.asarray(b1b), T, BQ)
    res = run_bass_kernel_spmd(nc, in_maps, list(range(N_CORES)))
    out = np.empty((B, T, 2 * H), dtype=np.float32)
    for q in range(N_Q):
        for d in range(2):
            h1 = res.results[2 * q + d]["h1out"]       # [NT, 128, KH*CH*BQ]
            h1 = h1.reshape(NT, 128, KH, CH, BQ)
            # [b, cc, j, k, p] -> [BQ, T, H]
            h1 = h1.transpose(4, 0, 3, 2, 1).reshape(BQ, T, H)
            if d == 1:
                h1 = h1[:, ::-1]
            out[q * BQ:(q + 1) * BQ, :, d * H:(d + 1) * H] = \
                h1.astype(np.float32)
    return out


# revision 26
# speedup vs baseline: 1.1628x; 1.1628x over previous
"""Trainium2 Bass kernel for a 2-layer stacked bidirectional LSTM.

Problem (hardcoded): B=64, T=512, D=512, H=512, 2 BiLSTM layers,
Keras gate order [i, f, g, o], sigmoid recurrent activation, tanh cell
activation, merge_mode='concat'.

Sharding: 8 cores = 2 directions x 4 batch quarters (B'=16 per core).

v2 design (all cores run the identical SPMD program, ALL in LOCAL time):
  - Host pre-reverses x in time for backward cores, packs weights with the
    gate order permuted to [i, f, o, g] (so sigmoid gates are contiguous),
    swaps W1's row blocks per direction (own-direction rows first), and
    un-reverses the output for backward cores.
  - Recurrence is fully unrolled in python (no hardware loops): 16 chunks
    of 32 steps per layer. zx (input projections) live only in SBUF; the
    projection matmuls for chunk cc+1 are interleaved between the
    recurrence steps of chunk cc so the PE stays dense (HAM warm) and has
    no DRAM zx roundtrip.
  - Gate PSUM: one bank holds the 12 sigmoid-gate m-chunks, another the 4
    tanh m-chunks (m-outer, k-inner accumulation groups). The sigmoid
    z-add + activation overlap the tanh-block matmuls (different banks).
  - Per step tail (critical path): tanh-z-add -> tanh -> ig -> c -> tanh(c)
    -> h, with f*c computed early in parallel.
  - Layer 0 h sequence goes to DRAM (local time), pairwise AllGather with
    the partner core, layer-1 projection reads own half straight and the
    partner half chunk-reversed (static) + step-reversed via gpsimd
    copies (negative-stride DMAs are rejected by walrus).
"""

import numpy as np
import ml_dtypes

import concourse.bass as bass
import concourse.mybir as mybir
import concourse.tile as tile
from concourse.bass import ds, ts
from concourse.bass_utils import run_bass_kernel_spmd

BF16 = mybir.dt.bfloat16
F32 = mybir.dt.float32
I32 = mybir.dt.int32
AF = mybir.ActivationFunctionType
ALU = mybir.AluOpType

# Problem dims (full size)
B_FULL, T_FULL, D_FULL, H_FULL = 64, 512, 512, 512
N_CORES = 8
N_Q = 4   # batch quarters; cores 2q (fwd) and 2q+1 (bwd) handle quarter q
CH = 32   # recurrence steps per chunk (CH*BQ = 512 tokens per chunk)

_MAXW = 1  # max sem-waits per instruction accepted by this walrus

# PE "heater" matmuls per recurrence step: independent matmuls into a dead
# PSUM bank that fill the PE-idle window while the serial gate/cell tail
# computes, so the HAM clock gate stays at K=8/8 (2.4 GHz) instead of
# oscillating back to 1.2 GHz between matmul bursts.
N_HEAT = 3
HEAT_N = 128


def _fix_walrus_compat(nc):
    """Adapt Tile-emitted IR to the deployed walrus:
    - drop EVENT_SEMAPHORE_RANGE_CLEAR (InstISA) kernel-tail cleanup,
    - split instructions carrying more than _MAXW semaphore waits into
      leading single-wait NOPs.
    """
    n_split = n_drop = 0
    for bb in nc.main_func.blocks:
        insts = bb.instructions
        out = []
        for inst in insts:
            if isinstance(inst, mybir.InstISA):
                n_drop += 1
                continue
            si = inst.sync_info
            if si is not None and len(si.on_wait) > _MAXW:
                waits = list(si.on_wait)
                extra, keep = waits[:-_MAXW], waits[-_MAXW:]
                for w in extra:
                    nop = mybir.InstNoOp(
                        name=nc.get_next_instruction_name(), ins=[], outs=[])
                    nop.engine = inst.engine
                    nop.sync_info = mybir.SyncInfo(on_wait=[w], on_update=[])
                    out.append(nop)
                    n_split += 1
                inst.sync_info = mybir.SyncInfo(
                    on_wait=keep, on_update=list(si.on_update))
            out.append(inst)
        insts[:] = out
    return n_drop, n_split


def build_program(T=T_FULL, BQ=B_FULL // N_Q, D=D_FULL, H=H_FULL,
                  single_core=False):
    G = 4 * H
    KD0 = D // 128           # k-chunks, layer-0 projection
    KD1 = 2 * H // 128       # k-chunks, layer-1 projection
    KH = H // 128            # k-chunks, recurrence
    MC = G // 128            # m-chunks of the gate dim
    MS = 3 * (MC // 4)       # sigmoid m-chunks (i, f, o)
    MT = MC // 4             # tanh m-chunks (g)
    NT = T // CH             # chunks per layer
    TW = CH * BQ             # tokens per chunk (= proj tile width)
    FH = KH * CH * BQ        # flattened h-chunk free size
    assert T % CH == 0 and D % 128 == 0 and H % 128 == 0

    nc = bass.Bass("TRN2", target_bir_lowering=False, debug=False,
                   num_devices=1 if single_core else N_CORES)

    # ---- I/O (all host-packed, local time, gate order [i,f,o,g]) ----
    x_in = nc.dram_tensor("x_in", [NT, 128, KD0 * TW], BF16,
                          kind="ExternalInput")
    w0 = nc.dram_tensor("w0", [128, KD0 * G], BF16, kind="ExternalInput")
    u0 = nc.dram_tensor("u0", [128, KH * G], BF16, kind="ExternalInput")
    b0 = nc.dram_tensor("b0", [128, MC], F32, kind="ExternalInput")
    w1 = nc.dram_tensor("w1", [128, KD1 * G], BF16, kind="ExternalInput")
    u1 = nc.dram_tensor("u1", [128, KH * G], BF16, kind="ExternalInput")
    b1 = nc.dram_tensor("b1", [128, MC], F32, kind="ExternalInput")
    pflag = nc.dram_tensor("pflag", [1, 1], I32, kind="ExternalInput")
    ident = nc.dram_tensor("ident", [128, 128], BF16, kind="ExternalInput")
    h1out = nc.dram_tensor("h1out", [NT, 128, FH], BF16,
                           kind="ExternalOutput")

    groups = [[2 * q, 2 * q + 1] for q in range(N_Q)]

    with tile.TileContext(nc) as tc:
        consts = tc.alloc_tile_pool(name="consts", bufs=1)
        dram = tc.alloc_tile_pool(name="dram", bufs=1, space="DRAM")

        # weights / biases resident in SBUF for the whole kernel
        w0_sb = consts.tile([128, KD0, G], BF16)
        nc.sync.dma_start(w0_sb, w0.ap())
        u0_sb = consts.tile([128, KH, G], BF16)
        nc.sync.dma_start(u0_sb, u0.ap())
        w1_sb = consts.tile([128, KD1, G], BF16)
        nc.sync.dma_start(w1_sb, w1.ap())
        u1_sb = consts.tile([128, KH, G], BF16)
        nc.sync.dma_start(u1_sb, u1.ap())
        b0_sb = consts.tile([128, MC], F32)
        nc.sync.dma_start(b0_sb, b0.ap())
        b1_sb = consts.tile([128, MC], F32)
        nc.sync.dma_start(b1_sb, b1.ap())
        pflag_sb = consts.tile([1, 1], I32)
        nc.sync.dma_start(pflag_sb, pflag.ap())
        id_sb = consts.tile([128, 128], BF16)
        nc.sync.dma_start(id_sb, ident.ap())
        zero_h = consts.tile([128, KH, BQ], BF16)
        nc.vector.memset(zero_h, 0.0)
        # cell state + tanh(g) scratch per layer: X[:, 0:KH] = tanh(g),
        # X[:, KH:2KH] = c
        X0 = consts.tile([128, 2 * KH, BQ], F32, name="X0")
        X1 = consts.tile([128, 2 * KH, BQ], F32, name="X1")
        nc.vector.memset(X0, 0.0)
        nc.vector.memset(X1, 0.0)

        # DRAM scratch: layer-0 h sequence (local time) + AllGather result
        # + partner's block (one dynamic-offset copy out of ag_out)
        h0loc = dram.tile([NT, 128, FH], BF16)
        ag_out = dram.tile([2 * NT, 128, FH], BF16)
        part = dram.tile([NT, 128, FH], BF16)

        fvp = nc.values_load(pflag_sb[0:1, 0:1], min_val=0, max_val=1)

        # ---------------- pools ----------------
        xpool = tc.alloc_tile_pool(name="xpool", bufs=2)
        zxpool = tc.alloc_tile_pool(name="zxpool", bufs=2)
        hckpool = tc.alloc_tile_pool(name="hckpool", bufs=2)
        tailpool = tc.alloc_tile_pool(name="tailpool", bufs=2)
        ps_sig = tc.alloc_tile_pool(name="ps_sig", bufs=1, space="PSUM")
        ps_tanh = tc.alloc_tile_pool(name="ps_tanh", bufs=1, space="PSUM")
        pp = tc.alloc_tile_pool(name="pp", bufs=2, space="PSUM")
        ps_heat = tc.alloc_tile_pool(name="ps_heat", bufs=1, space="PSUM")

        psig = ps_sig.tile([128, MS, BQ], F32, name="psig")
        ptanh = ps_tanh.tile([128, MT, BQ], F32, name="ptanh")
        pheat = ps_heat.tile([128, HEAT_N], F32, name="pheat")

        def heaters(u_sb):
            for _ in range(N_HEAT):
                nc.tensor.matmul(pheat, u_sb[:, 0, 0:128],
                                 u_sb[:, 1, 0:HEAT_N], start=True, stop=True)

        # ---------------- helpers ----------------
        def load_x0(cc, xs):
            """x tile for layer-0 projection of chunk cc."""
            nc.sync.dma_start(
                xs, x_in.ap()[cc].rearrange("p (k t) -> p k t", k=KD0))

        def proj_group(m, w_sb, x_sb, KD, zx_sb, b_sb):
            """One projection m-group: 4|8 matmuls (N=TW) + biased copy."""
            ps = pp.tile([128, TW], F32, tag="pp")
            for k in range(KD):
                nc.tensor.matmul(ps, w_sb[:, k, ts(m, 128)], x_sb[:, k, :],
                                 start=(k == 0), stop=(k == KD - 1))
            nc.scalar.activation(
                zx_sb[:, :, m, :],
                ps.rearrange("p (j b) -> p j b", j=CH),
                AF.Identity, bias=b_sb[:, m:m + 1])

        def load_x1(cc, xs, pt):
            """Layer-1 proj inputs for chunk cc: own half straight from
            h0loc[cc]; partner half = part[NT-1-cc] (chunk-reversed)."""
            nc.sync.dma_start(
                xs[:, 0:KH, :],
                h0loc[cc].rearrange("p (k t) -> p k t", k=KH))
            nc.sync.dma_start(pt[:, 0, :], part[NT - 1 - cc])

        def reverse_steps(pt, xs, j_lo, j_hi):
            """Step-reverse partner h chunk into xs[:, KH:2KH, :]."""
            ptv = pt.rearrange("p one (k j b) -> p one k j b", k=KH, j=CH)
            for j in range(j_lo, j_hi):
                nc.gpsimd.tensor_copy(
                    xs[:, KH:2 * KH, ts(CH - 1 - j, BQ)],
                    ptv[:, 0, :, j, :])

        # ---------------- one recurrence step ----------------
        def step(u_sb, zx_sb, j, rhs_tile, rhs_j, h_ck, X):
            """rhs = rhs_tile[:, k, rhs_j, :] ([128,KH,CH,BQ]) or zero_h
            ([128,KH,BQ]) when rhs_j is None."""
            def rhs(k):
                if rhs_j is None:
                    return rhs_tile[:, k, :]
                return rhs_tile[:, k, rhs_j, :]

            # zx goes into PSUM first via an identity matmul (start=True
            # sets has_written for the whole region), then the U.h m-groups
            # accumulate on top with start=False.
            nc.tensor.matmul(
                psig, id_sb,
                zx_sb[:, j, 0:MS, :].rearrange("p m b -> p (m b)"),
                start=True, stop=False, skip_group_check=True)
            for m in range(MS):
                for k in range(KH):
                    nc.tensor.matmul(psig[:, m, :], u_sb[:, k, ts(m, 128)],
                                     rhs(k), start=False,
                                     stop=(m == MS - 1 and k == KH - 1),
                                     skip_group_check=True)
            # sigmoid block complete: activation reads PSUM directly and
            # overlaps the tanh-block matmuls (different PSUM banks)
            S = tailpool.tile([128, MS, BQ], F32, tag="S")
            nc.scalar.activation(S, psig, AF.Sigmoid)
            nc.tensor.matmul(
                ptanh, id_sb,
                zx_sb[:, j, MS:MC, :].rearrange("p m b -> p (m b)"),
                start=True, stop=False, skip_group_check=True)
            for m in range(MT):
                for k in range(KH):
                    nc.tensor.matmul(ptanh[:, m, :],
                                     u_sb[:, k, ts(MS + m, 128)],
                                     rhs(k), start=False,
                                     stop=(m == MT - 1 and k == KH - 1),
                                     skip_group_check=True)
            # f*c early (off the critical chain): fc = S[f] * X[c]
            fc = tailpool.tile([128, KH, BQ], F32, tag="fc")
            nc.vector.tensor_tensor(
                fc, S[:, KH:2 * KH, :], X[:, KH:2 * KH, :], ALU.mult)
            # critical chain: tanh(g) -> ig -> c -> tanh(c) -> h
            nc.scalar.activation(X[:, 0:KH, :], ptanh, AF.Tanh)
            ig = tailpool.tile([128, KH, BQ], F32, tag="ig")
            nc.vector.tensor_tensor(
                ig, S[:, 0:KH, :], X[:, 0:KH, :], ALU.mult)
            nc.vector.tensor_tensor(X[:, KH:2 * KH, :], ig, fc, ALU.add)
            th = tailpool.tile([128, KH, BQ], F32, tag="th")
            nc.scalar.activation(th, X[:, KH:2 * KH, :], AF.Tanh)
            nc.vector.tensor_tensor(
                h_ck[:, :, j, :], S[:, 2 * KH:3 * KH, :], th, ALU.mult)

        # ================= layer 0 =================
        x_sb = {}
        x_sb[0] = xpool.tile([128, KD0, TW], BF16, tag="x0", name="xsb0")
        load_x0(0, x_sb[0])
        zx = {}
        zx[0] = zxpool.tile([128, CH, MC, BQ], BF16, tag="zx", name="zx0")
        for m in range(MC):
            proj_group(m, w0_sb, x_sb[0], KD0, zx[0], b0_sb)
        x_sb[1] = xpool.tile([128, KD0, TW], BF16, tag="x0", name="xsb1")
        load_x0(1, x_sb[1])

        h_ck_prev = None
        for cc in range(NT):
            h_ck = hckpool.tile([128, KH, CH, BQ], BF16, tag="hck0")
            if cc + 1 < NT:
                zx[cc + 1] = zxpool.tile([128, CH, MC, BQ], BF16, tag="zx", name=f"zx{cc+1}")
            for j in range(CH):
                if j == 0:
                    if cc == 0:
                        step(u0_sb, zx[cc], j, zero_h, None, h_ck, X0)
                    else:
                        step(u0_sb, zx[cc], j, h_ck_prev, CH - 1, h_ck, X0)
                else:
                    step(u0_sb, zx[cc], j, h_ck, j - 1, h_ck, X0)
                if j == 1 and cc + 2 < NT:
                    x_sb[cc + 2] = xpool.tile([128, KD0, TW], BF16, tag="x0", name=f"xsb{cc+2}")
                    load_x0(cc + 2, x_sb[cc + 2])
                if j % 2 == 0 and cc + 1 < NT:
                    proj_group(j // 2, w0_sb, x_sb[cc + 1], KD0,
                               zx[cc + 1], b0_sb)
                heaters(u0_sb)
            nc.sync.dma_start(h0loc[cc],
                              h_ck.rearrange("p k j b -> p (k j b)"))
            h_ck_prev = h_ck

        # ================= AllGather =================
        if single_core:
            nc.sync.dma_start(ag_out[0:NT], h0loc)
            nc.sync.dma_start(ag_out[NT:2 * NT], h0loc)
        else:
            nc.gpsimd.collective_compute(
                "AllGather", ALU.bypass, replica_groups=groups,
                ins=[h0loc.opt()], outs=[ag_out.opt()])
        # copy the partner's whole block out of ag_out with a single
        # dynamic-offset DMA; everything downstream is then static
        poff = nc.s_assert_within(fvp * (NT * 128 * FH), 0, NT * 128 * FH)
        apg = ag_out[0]
        nc.sync.dma_start(
            part.rearrange("c p f -> (c p) f"),
            bass.AP(tensor=apg.tensor, offset=apg.offset + poff,
                    ap=[[FH, NT * 128], [1, FH]]))

        # ================= layer 1 =================
        x1 = {}
        pt = {}
        x1[0] = xpool.tile([128, KD1, TW], BF16, tag="x1", name="x1_0")
        pt[0] = xpool.tile([128, 1, FH], BF16, tag="pt", name="pt0")
        load_x1(0, x1[0], pt[0])
        reverse_steps(pt[0], x1[0], 0, CH)
        zx1 = {}
        zx1[0] = zxpool.tile([128, CH, MC, BQ], BF16, tag="zx", name="zx1_0")
        for m in range(MC):
            proj_group(m, w1_sb, x1[0], KD1, zx1[0], b1_sb)

        h_ck_prev = None
        for cc in range(NT):
            h_ck = hckpool.tile([128, KH, CH, BQ], BF16, tag="hck1")
            if cc + 1 < NT:
                zx1[cc + 1] = zxpool.tile([128, CH, MC, BQ], BF16, tag="zx", name=f"zx1_{cc+1}")
            for j in range(CH):
                if j == 0:
                    if cc == 0:
                        step(u1_sb, zx1[cc], j, zero_h, None, h_ck, X1)
                    else:
                        step(u1_sb, zx1[cc], j, h_ck_prev, CH - 1, h_ck, X1)
                else:
                    step(u1_sb, zx1[cc], j, h_ck, j - 1, h_ck, X1)
                if cc + 1 < NT:
                    if j == 0:
                        x1[cc + 1] = xpool.tile([128, KD1, TW], BF16,
                                                tag="x1", name=f"x1_{cc+1}")
                        pt[cc + 1] = xpool.tile([128, 1, FH], BF16, tag="pt", name=f"pt{cc+1}")
                        load_x1(cc + 1, x1[cc + 1], pt[cc + 1])
                    if 2 <= j < 10:
                        reverse_steps(pt[cc + 1], x1[cc + 1],
                                      (j - 2) * 4, (j - 1) * 4)
                    if 10 <= j < 26:
                        proj_group(j - 10, w1_sb, x1[cc + 1], KD1,
                                   zx1[cc + 1], b1_sb)
                heaters(u1_sb)
            nc.sync.dma_start(h1out.ap()[cc],
                              h_ck.rearrange("p k j b -> p (k j b)"))
            h_ck_prev = h_ck

        for p in (ps_heat, pp, ps_tanh, ps_sig, tailpool, hckpool, zxpool, xpool,
                  dram, consts):
            p.release()

    _fix_walrus_compat(nc)
    return nc


# gate permutation: Keras [i, f, g, o] -> kernel [i, f, o, g]
def _gate_perm(H):
    return np.concatenate([np.arange(0, H), np.arange(H, 2 * H),
                           np.arange(3 * H, 4 * H), np.arange(2 * H, 3 * H)])


def _pack_w(W, H):
    """[Din, G] -> [128, KD*G] bf16 with gate perm; row-major k-chunks."""
    bf = ml_dtypes.bfloat16
    Din, G = W.shape
    Wp = W[:, _gate_perm(H)]
    KD = Din // 128
    # [KD, 128, G] -> [128, KD, G]
    return np.ascontiguousarray(
        Wp.reshape(KD, 128, G).transpose(1, 0, 2).reshape(128, KD * G)
    ).astype(bf)


def _pack_b(b, H):
    MC = b.shape[0] // 128
    bp = b[_gate_perm(H)]
    return np.ascontiguousarray(bp.reshape(MC, 128).T).astype(np.float32)


def _prep_core_inputs(x, W0f, U0f, b0f, W0b, U0b, b0b,
                      W1f, U1f, b1f, W1b, U1b, b1b, T, BQ):
    """Host-side sharding: list of 8 input dicts (core = 2q + dir)."""
    bf = ml_dtypes.bfloat16
    B, _, D = x.shape
    H = U0f.shape[0]
    NT = T // CH
    KD0 = D // 128
    Wd = {0: (W0f, U0f, b0f, W1f, U1f, b1f),
          1: (W0b, U0b, b0b, W1b, U1b, b1b)}
    packed = {}
    for d in range(2):
        W0, U0, b0, W1, U1, b1 = Wd[d]
        # W1 rows: own-direction block first, partner block second
        if d == 0:
            W1o = W1
        else:
            W1o = np.concatenate([W1[H:2 * H], W1[0:H]], axis=0)
        packed[d] = {
            "w0": _pack_w(np.asarray(W0), H),
            "u0": _pack_w(np.asarray(U0), H),
            "b0": _pack_b(np.asarray(b0), H),
            "w1": _pack_w(np.asarray(W1o), H),
            "u1": _pack_w(np.asarray(U1), H),
            "b1": _pack_b(np.asarray(b1), H),
            "pflag": np.array([[1 - d]], dtype=np.int32),
            "ident": np.eye(128, dtype=np.float32).astype(bf),
        }
    in_maps = []
    for q in range(N_Q):
        xq = np.asarray(x[q * BQ:(q + 1) * BQ])      # [BQ, T, D]
        for d in range(2):
            xl = xq if d == 0 else xq[:, ::-1]
            # x_in[cc, p, kd, j*BQ+b] = xl[b, cc*CH+j, kd*128+p]
            xi = (xl.transpose(2, 1, 0)               # [D, T, BQ]
                  .reshape(KD0, 128, NT, CH, BQ)
                  .transpose(2, 1, 0, 3, 4)           # [NT, 128, KD0, CH, BQ]
                  .reshape(NT, 128, KD0 * CH * BQ))
            in_maps.append({
                "x_in": np.ascontiguousarray(xi).astype(bf),
                **packed[d],
            })
    return in_maps


def kernel(x, W0f, U0f, b0f, W0b, U0b, b0b,
           W1f, U1f, b1f, W1b, U1b, b1b):
    x = np.asarray(x, dtype=np.float32)
    B, T, D = x.shape
    H = U0f.shape[0]
    BQ = B // N_Q
    NT = T // CH
    KH = H // 128
    nc = build_program(T=T, BQ=BQ, D=D, H=H)
    in_maps = _prep_core_inputs(
        x, np.asarray(W0f), np.asarray(U0f), np.asarray(b0f),
        np.asarray(W0b), np.asarray(U0b), np.asarray(b0b),
        np.asarray(W1f), np.asarray(U1f), np.asarray(b1f),
        np.asarray(W1b), np.asarray(U1b), np.asarray(b1b), T, BQ)
    res = run_bass_kernel_spmd(nc, in_maps, list(range(N_CORES)))
    out = np.empty((B, T, 2 * H), dtype=np.float32)
    for q in range(N_Q):
        for d in range(2):
            h1 = res.results[2 * q + d]["h1out"]       # [NT, 128, KH*CH*BQ]
            h1 = h1.reshape(NT, 128, KH, CH, BQ)
            # [b, cc, j, k, p] -> [BQ, T, H]
            h1 = h1.transpose(4, 0, 3, 2, 1).reshape(BQ, T, H)
            if d == 1:
                h1 = h1[:, ::-1]
            out[q * BQ:(q + 1) * BQ, :, d * H:(d + 1) * H] = \
                h1.astype(np.float32)
    return out
